# revision 24
# baseline (speedup 1.0000x reference)
"""Self-contained TRN2 Bass kernel for causal multi-head attention.

Problem: x[4,2048,1024], causal mask, wq/wk/wv/wo [1024,1024], H=16, HD=64.
Sharding: 8 NeuronCores = 4 batches x 2 head-groups (8 heads each).
Each core computes Q/K/V projections for its head group, causal attention
(block-skipped via host mask classification), and a partial o_proj; the two
partial outputs per batch are summed on host (the tensor-parallel
all-reduce of the unshard step).

v5: per-block fused schedule. Only Q/K/V of token-block 0 run up front
(~20us); the projections of block tb+1 and the o_proj of block ib-1 are
"fillers" interleaved into attention(ib)'s head-pair loops, keeping the PE
busy through every exp-latency window while the Act engine streams the
softmax exps nearly end-to-end. Everything is bf16 except PSUM accumulation
and the softmax normalization. One PSUM plan serves the whole kernel:
a 2-deep [128,1024] ring (score tiles, o_proj psy tiles, projection
pair-chunks) + 4 single-bank po tiles (two alternating pairs of packed-head
AV accumulators). Inputs are packed so each weight/x-block/mask set is one
DMA. Mixed causal blocks skip their fully-masked leading query columns in
QK/exp/mask/AV (diagonal trimming).
"""
import sys
sys.path.insert(0, "/opt/trn_rl_repo")

import numpy as np
import ml_dtypes

from contextlib import ExitStack

import concourse.bass as bass
import concourse.mybir as mybir
import concourse.tile as tile
from concourse import bacc

f32 = mybir.dt.float32
bf16 = mybir.dt.bfloat16
EXP = mybir.ActivationFunctionType.Exp


def build(T=2048, C=1024, HL=8, D=64, kinds=None, mixpat=None, mixtrim=None,
          npat=0, n_cores=8, debug=False):
    OL = HL * D
    JT = T // 128
    IBN = T // 512
    NC = C // 128
    NO = OL // 128
    NTB = T // 512
    NP = HL // 2
    SCALE = 1.0 / float(D) ** 0.5
    E = D + 1  # v columns per head incl. ones column

    assert kinds is not None

    nc = bacc.Bacc("TRN2", target_bir_lowering=False, debug=False,
                   num_devices=n_cores)

    xT = nc.dram_tensor("xT", [C, T], bf16, kind="ExternalInput").ap()
    wqT = nc.dram_tensor("wqT", [C, OL], bf16, kind="ExternalInput").ap()
    wkT = nc.dram_tensor("wkT", [C, OL], bf16, kind="ExternalInput").ap()
    wvT = nc.dram_tensor("wvT", [C, OL], bf16, kind="ExternalInput").ap()
    woT = nc.dram_tensor("woT", [OL, C], bf16, kind="ExternalInput").ap()
    maskT = None
    if npat:
        maskT = nc.dram_tensor("maskT", [npat, 128, 1024], bf16,
                               kind="ExternalInput").ap()
    y = nc.dram_tensor("y", [T, C], f32, kind="ExternalOutput").ap()
    dbg = {}
    if debug:
        dbg["qT"] = nc.dram_tensor("dbg_qT", [NO, 128, T], bf16, kind="ExternalOutput").ap()
        dbg["kT"] = nc.dram_tensor("dbg_kT", [NO, 128, T], bf16, kind="ExternalOutput").ap()
        dbg["v"] = nc.dram_tensor("dbg_v", [JT, 128, HL * E], bf16, kind="ExternalOutput").ap()
        dbg["aT"] = nc.dram_tensor("dbg_aT", [NO, 128, T], bf16, kind="ExternalOutput").ap()

    with tile.TileContext(nc) as tc, ExitStack() as ctx:
        # ---- pools live for the whole kernel ----
        persist = ctx.enter_context(tc.tile_pool(name="persist", bufs=1))
        ppp = ctx.enter_context(tc.tile_pool(name="ppp", bufs=6))
        pnrm = ctx.enter_context(tc.tile_pool(name="pnrm", bufs=2))
        pys = ctx.enter_context(tc.tile_pool(name="pys", bufs=3))
        psS = ctx.enter_context(tc.tile_pool(name="psS", bufs=2, space="PSUM"))
        po = ctx.enter_context(tc.tile_pool(name="po", bufs=1, space="PSUM"))

        qT_sb = [persist.tile([128, T], bf16, tag=f"qT{o}", name=f"qT{o}")
                 for o in range(NO)]
        kT_sb = [persist.tile([128, T], bf16, tag=f"kT{o}", name=f"kT{o}")
                 for o in range(NO)]
        v_sb = [persist.tile([128, HL * E], bf16, tag=f"v{t}", name=f"v{t}")
                for t in range(JT)]
        aT_sb = [persist.tile([128, T], bf16, tag=f"aT{o}", name=f"aT{o}")
                 for o in range(NO)]
        # packed: one DMA per tensor
        wq_all = persist.tile([128, NC, OL], bf16, tag="wq", name="wq_all")
        wk_all = persist.tile([128, NC, OL], bf16, tag="wk", name="wk_all")
        wv_all = persist.tile([128, NC, OL], bf16, tag="wv", name="wv_all")
        wo_all = persist.tile([128, NO, C], bf16, tag="wo", name="wo_all")
        msk_all = None
        if npat:
            msk_all = persist.tile([128, npat, 1024], bf16, tag="msk",
                                   name="msk_all")
        x_all = [persist.tile([128, NC, 512], bf16, tag=f"x{tb}",
                              name=f"x_all{tb}") for tb in range(NTB)]

        # x(0) + wk feed the first compute: split into chunks so several DMA
        # engines run in parallel; later loads are latency-insensitive
        x0r = xT[:, 0:512].rearrange("(c p) t -> p c t", p=128)
        wkr = wkT.rearrange("(c p) o -> p c o", p=128)
        for h in range(4):
            cs = slice(2 * h, 2 * h + 2)
            nc.sync.dma_start(x_all[0][:, cs, :], x0r[:, cs, :])
            nc.sync.dma_start(wk_all[:, cs, :], wkr[:, cs, :])
        wvr = wvT.rearrange("(c p) o -> p c o", p=128)
        wqr = wqT.rearrange("(c p) o -> p c o", p=128)
        for h in range(2):
            cs = slice(4 * h, 4 * h + 4)
            nc.sync.dma_start(wv_all[:, cs, :], wvr[:, cs, :])
            nc.sync.dma_start(wq_all[:, cs, :], wqr[:, cs, :])
        for tb in range(1, NTB):
            xr = xT[:, tb * 512:(tb + 1) * 512].rearrange("(c p) t -> p c t", p=128)
            for h in range(2):
                cs = slice(4 * h, 4 * h + 4)
                nc.sync.dma_start(x_all[tb][:, cs, :], xr[:, cs, :])
        nc.sync.dma_start(wo_all[:], woT.rearrange("(o p) c -> p o c", p=128))
        if npat:
            nc.sync.dma_start(msk_all[:], maskT.rearrange("s p q -> p s q"))
        for t in range(JT):
            nc.vector.memset(v_sb[t][:], 1.0)

        def proj_pair(w_all, dst, o0, tb, kind):
            """Two [128,512] projection chunks (o0, o0+1) in one ring slot."""
            tbs = slice(tb * 512, (tb + 1) * 512)
            ps = psS.tile([128, 1024], f32, tag="sS", name=f"pj_{kind}_{o0}_{tb}")
            for half in range(2):
                o = o0 + half
                hp = ps[:, half * 512:(half + 1) * 512]
                for c in range(NC):
                    nc.tensor.matmul(hp, w_all[:, c, o * 128:(o + 1) * 128],
                                     x_all[tb][:, c, :], start=(c == 0),
                                     stop=(c == NC - 1))
            for half in range(2):
                o = o0 + half
                hp = ps[:, half * 512:(half + 1) * 512]
                nc.vector.tensor_copy(dst[o][:, tbs], hp)

        def vproj_pair(sub0, tb):
            """Two V chunks (sub0, sub0+1): psv [128 t, 512 od] halves."""
            ps = psS.tile([128, 1024], f32, tag="sS", name=f"pj_v_{sub0}_{tb}")
            for half in range(2):
                sub = sub0 + half
                hp = ps[:, half * 512:(half + 1) * 512]
                for c in range(NC):
                    nc.tensor.matmul(hp, x_all[tb][:, c, sub * 128:(sub + 1) * 128],
                                     wv_all[:, c, :], start=(c == 0),
                                     stop=(c == NC - 1))
            for half in range(2):
                t = tb * 4 + sub0 + half
                hp = ps[:, half * 512:(half + 1) * 512]
                v3 = v_sb[t][:].rearrange("p (h e) -> p h e", e=E)
                ps3 = hp.rearrange("p (h e) -> p h e", e=D)
                nc.vector.tensor_copy(v3[:, :, 0:D], ps3)

        def emit_oproj(qc, direct=False):
            qq = slice(qc * 128, (qc + 1) * 128)
            psy = psS.tile([128, C], f32, tag="sS", name=f"psy_{qc}")
            for half in range(2):
                cs = slice(half * 512, (half + 1) * 512)
                for o in range(NO):
                    nc.tensor.matmul(psy[:, cs], aT_sb[o][:, qq],
                                     wo_all[:, o, cs],
                                     start=(o == 0), stop=(o == NO - 1))
            ys = pys.tile([128, C], f32, tag="ys", name=f"ys_{qc}")
            if direct:
                # tail chunks: spread copies/DMAs across engines and queues so
                # the drain after the last matmul pipelines
                if qc % 2:
                    nc.scalar.copy(ys[:], psy[:])
                    nc.gpsimd.dma_start(y[qq, :], ys[:])
                else:
                    nc.vector.tensor_copy(ys[:], psy[:])
                    nc.sync.dma_start(y[qq, :], ys[:])
            else:
                nc.vector.tensor_copy(ys[:], psy[:])
                nc.gpsimd.dma_start(y[qq, :], ys[:])

        def run_filler(fl):
            if fl[0] == "psy":
                emit_oproj(fl[1])
            elif fl[0] == "q":
                proj_pair(wq_all, qT_sb, fl[1], fl[2], "q")
            elif fl[0] == "k":
                proj_pair(wk_all, kT_sb, fl[1], fl[2], "k")
            else:
                vproj_pair(fl[1], fl[2])

        # ======== front: K/V/Q of block 0 only ========
        for fl in [("k", 0, 0), ("k", 2, 0), ("v", 0, 0), ("v", 2, 0),
                   ("q", 0, 0), ("q", 2, 0)]:
            run_filler(fl)

        # ======== attention, everything else interleaved as fillers ========
        pending = []   # o_proj 128-query chunks awaiting emission
        for ib in range(IBN):
            js = [jt for jt in range(JT) if kinds[jt][ib] != 0]
            nj = len(js)
            fillq = []
            if ib + 1 < NTB:
                fillq.extend([("q", 0, ib + 1), ("q", 2, ib + 1),
                              ("k", 0, ib + 1), ("k", 2, ib + 1),
                              ("v", 0, ib + 1), ("v", 2, ib + 1)])
            # defer most o_proj chunks toward the last (longest, exp-paced)
            # block rows, where the PE has exp-latency bubbles to fill
            take = 2 if ib + 1 < IBN else len(pending)
            fillq.extend(("psy", qc) for qc in pending[:take])
            del pending[:take]

            for p in range(NP):
                hA, hB = 2 * p, 2 * p + 1
                poA = po.tile([E, 512], f32, tag=f"poA{p % 2}",
                              name=f"poA_{ib}_{p}")
                poB = po.tile([E, 512], f32, tag=f"poB{p % 2}",
                              name=f"poB_{ib}_{p}")

                trims = []
                for jt in js:
                    tr = mixtrim[jt][ib] if kinds[jt][ib] == 2 else 0
                    trims.append(tr)
                trims[0] = 0  # first block must cover the full accumulator

                def emit_qk(cidx):
                    jt = js[cidx]
                    tr = trims[cidx]
                    jj = slice(jt * 128, (jt + 1) * 128)
                    qq = slice(ib * 512 + tr, (ib + 1) * 512)
                    sAB = psS.tile([128, 1024], f32, tag="sS",
                                   name=f"s_{ib}_{p}_{cidx}")
                    nc.tensor.matmul(sAB[:, tr:512], kT_sb[p][0:64, jj],
                                     qT_sb[p][0:64, qq],
                                     start=True, stop=True,
                                     tile_position=(0, 0))
                    nc.tensor.matmul(sAB[:, 512 + tr:1024], kT_sb[p][64:128, jj],
                                     qT_sb[p][64:128, qq],
                                     start=True, stop=True,
                                     tile_position=(64, 0))
                    pAB = ppp.tile([128, 1024], bf16, tag="pP",
                                   name=f"pp_{ib}_{p}_{cidx}")
                    if tr:
                        s3 = sAB.rearrange("p (h q) -> p h q", q=512)
                        p3 = pAB[:].rearrange("p (h q) -> p h q", q=512)
                        nc.scalar.activation(p3[:, :, tr:512], s3[:, :, tr:512],
                                             EXP, scale=SCALE)
                    else:
                        nc.scalar.activation(pAB[:], sAB[:], EXP, scale=SCALE)
                    if kinds[jt][ib] == 2:
                        p3 = pAB[:].rearrange("p (h q) -> p h q", q=512)
                        m3 = msk_all[:, mixpat[jt][ib], :].rearrange(
                            "p (h q) -> p h q", q=512)
                        nc.vector.tensor_mul(p3[:, :, tr:512], p3[:, :, tr:512],
                                             m3[:, :, tr:512])
                    return pAB

                def emit_av(cidx, pAB):
                    jt = js[cidx]
                    tr = trims[cidx]
                    first = (cidx == 0)
                    last = (cidx == nj - 1)
                    nc.tensor.matmul(poA[:, tr:512],
                                     v_sb[jt][:, hA * E:(hA + 1) * E],
                                     pAB[:, tr:512], start=first, stop=last,
                                     skip_group_check=True)
                    nc.tensor.matmul(poB[:, tr:512],
                                     v_sb[jt][:, hB * E:(hB + 1) * E],
                                     pAB[:, 512 + tr:1024], start=first,
                                     stop=last, skip_group_check=True)

                # software pipeline: QK0, QK1, filler, AV0, QK2, AV1, ...
                # extra fillers every few blocks keep PE ahead of the exp
                # stream on long rows
                pabs = {}
                pabs[0] = emit_qk(0)
                if nj > 1:
                    pabs[1] = emit_qk(1)
                nfill = 2 if nj > 8 else 1
                for _ in range(nfill):
                    if fillq:
                        run_filler(fillq.pop(0))
                for cidx in range(nj):
                    if cidx + 2 < nj:
                        pabs[cidx + 2] = emit_qk(cidx + 2)
                    if cidx and cidx % 5 == 0 and fillq:
                        run_filler(fillq.pop(0))
                    emit_av(cidx, pabs.pop(cidx))
                # softmax normalization: denominators sit in row 64 (ones col)
                dnA = pnrm.tile([1, 512], f32, tag="dnA", name=f"dnA_{ib}_{p}")
                dnB = pnrm.tile([1, 512], f32, tag="dnB", name=f"dnB_{ib}_{p}")
                nc.vector.tensor_copy(dnA[:], poA[64:65, :])
                nc.vector.tensor_copy(dnB[:], poB[64:65, :])
                rrA = pnrm.tile([1, 512], f32, tag="rrA", name=f"rrA_{ib}_{p}")
                rrB = pnrm.tile([1, 512], f32, tag="rrB", name=f"rrB_{ib}_{p}")
                nc.vector.reciprocal_approx_fast(rrA[:], dnA[:])
                nc.vector.reciprocal_approx_fast(rrB[:], dnB[:])
                bcA = pnrm.tile([64, 512], f32, tag="bcA", name=f"bcA_{ib}_{p}")
                bcB = pnrm.tile([64, 512], f32, tag="bcB", name=f"bcB_{ib}_{p}")
                nc.gpsimd.partition_broadcast(bcA[:], rrA[:])
                nc.gpsimd.partition_broadcast(bcB[:], rrB[:])
                ii = slice(ib * 512, (ib + 1) * 512)
                nc.vector.tensor_mul(aT_sb[p][0:64, ii], poA[0:64, :], bcA[:])
                stgB = pnrm.tile([64, 512], bf16, tag="stgB", name=f"stgB_{ib}_{p}")
                nc.vector.tensor_mul(stgB[:], poB[0:64, :], bcB[:])
                nc.gpsimd.dma_start(aT_sb[p][64:128, ii], stgB[:])
            for fl in fillq:   # drain any leftovers
                run_filler(fl)
            pending.extend(range(ib * 4, ib * 4 + 4))
        for qc in pending:
            emit_oproj(qc, direct=True)

        if debug:
            for o in range(NO):
                nc.sync.dma_start(dbg["qT"][o], qT_sb[o][:])
                nc.sync.dma_start(dbg["kT"][o], kT_sb[o][:])
                nc.sync.dma_start(dbg["aT"][o], aT_sb[o][:])
            for t in range(JT):
                nc.sync.dma_start(dbg["v"][t], v_sb[t][:])

    nc.compile()
    return nc


def classify_mask(mask2d, T):
    """mask2d: [T, T] (i=query rows, j=key cols).

    Returns kinds[jt][ib] in {0 empty, 1 full, 2 mixed}, mixpat[jt][ib]
    (index into the deduped pattern list), mixtrim[jt][ib] (count of leading
    query columns that are entirely masked, so QK/exp/AV can skip them), and
    patterns [n, 128, 1024] float32 (key-major tiles, duplicated along the
    free axis so one multiply covers both packed heads)."""
    JT, IBN = T // 128, T // 512
    kinds = [[0] * IBN for _ in range(JT)]
    mixpat = [[-1] * IBN for _ in range(JT)]
    mixtrim = [[0] * IBN for _ in range(JT)]
    patterns = []
    seen = {}
    for jt in range(JT):
        for ib in range(IBN):
            blk = mask2d[ib * 512:(ib + 1) * 512, jt * 128:(jt + 1) * 128]
            if not blk.any():
                kinds[jt][ib] = 0
            elif blk.all():
                kinds[jt][ib] = 1
            else:
                kinds[jt][ib] = 2
                tileT = np.ascontiguousarray(blk.T.astype(np.float32))
                key = tileT.tobytes()
                if key not in seen:
                    seen[key] = len(patterns)
                    patterns.append(np.concatenate([tileT, tileT], axis=1))
                mixpat[jt][ib] = seen[key]
                colvalid = tileT.any(axis=0)
                mixtrim[jt][ib] = int(np.argmax(colvalid))
    pat = np.stack(patterns) if patterns else None
    return kinds, mixpat, mixtrim, pat


B, T, C = 4, 2048, 1024
H, HD = 16, 64
G = 2
HL = H // G
OL = HL * HD

_cache = {}


def _prepare(x, mask, wq, wk, wv, wo):
    """Classify the mask, build (or reuse) the compiled kernel, and build
    the 8 per-core input maps."""
    bf = ml_dtypes.bfloat16
    x = np.asarray(x, dtype=np.float32)
    mask = np.asarray(mask)
    wq = np.asarray(wq, dtype=np.float32)
    wk = np.asarray(wk, dtype=np.float32)
    wv = np.asarray(wv, dtype=np.float32)
    wo = np.asarray(wo, dtype=np.float32)

    mask2d = mask.reshape(mask.shape[-2], mask.shape[-1])
    kinds, mixpat, mixtrim, pat = classify_mask(mask2d, T)
    npat = 0 if pat is None else len(pat)
    pat_bf = None if pat is None else pat.astype(bf)

    key = (tuple(tuple(r) for r in kinds), tuple(tuple(r) for r in mixpat),
           tuple(tuple(r) for r in mixtrim))
    if key not in _cache:
        _cache[key] = build(T=T, C=C, HL=HL, D=HD, kinds=kinds, mixpat=mixpat,
                            mixtrim=mixtrim, npat=npat, n_cores=8)
    nc = _cache[key]

    in_maps = []
    for b in range(B):
        for g in range(G):
            m = {
                "xT": np.ascontiguousarray(x[b].T).astype(bf),
                "wqT": np.ascontiguousarray(wq[g * OL:(g + 1) * OL, :].T).astype(bf),
                "wkT": np.ascontiguousarray(wk[g * OL:(g + 1) * OL, :].T).astype(bf),
                "wvT": np.ascontiguousarray(wv[g * OL:(g + 1) * OL, :].T).astype(bf),
                "woT": np.ascontiguousarray(wo[:, g * OL:(g + 1) * OL].T).astype(bf),
            }
            if npat:
                m["maskT"] = pat_bf
            in_maps.append(m)
    return nc, in_maps


def _gather(results):
    out = np.empty((B, T, C), np.float32)
    for b in range(B):
        out[b] = results[2 * b]["y"] + results[2 * b + 1]["y"]
    return out


def kernel(x, mask, wq, wk, wv, wo):
    from concourse import bass_utils
    nc, in_maps = _prepare(x, mask, wq, wk, wv, wo)
    res = bass_utils.run_bass_kernel_spmd(nc, in_maps, core_ids=list(range(8)))
    return _gather(res.results)
